# revision 6
# baseline (speedup 1.0000x reference)
"""Trainium2 Bass kernel for nn_CapsuleLayer (conv capsule layer with dynamic routing).

Full (unsharded) inputs in, full output out. Sharding: data-parallel over the
num_capsules axis A=32 -> 8 cores x 4 capsules each (x windows replicated).

Per-core algorithm (all in SBUF, nothing big spilled to HBM):
  - host does the im2col: xw[p, n, c], p = (b,i,j) of 576 positions, n = 288
    route nodes (ci,ki,kj), c = 8 input channels.
  - priors[p,n,d] = sum_c xw[p,n,c] W[a,n,c,d] computed on the PE as 18
    block-diagonal matmuls per (a, position-tile): stationary = xw^T chunk
    [(g,c)=128, p<=128], moving = block-diag W [(g,c)=128, (g,d)=256].
  - routing iterations fused on DVE/ACT with priors held in SBUF fp16:
      iter0: probs uniform -> s0 = (1/N) sum_nc xw.W  (dense PE matmul, PSUM acc)
      g[p,n] = sum_d priors*out  (fp16 tensor_tensor + X-reduce)
      softmax via exp(logits-max) on ACT with fused accum sum; the 1/sumexp is
      applied after the weighted sum (linearity) so probs are never materialized
      s[p,d] = sum_n e*priors    (fp16 tensor_tensor + X-reduce on a transposed
                                  view) then scale+bias in one scalar_tensor_tensor
      squash in f32 (tiny [p,16] ops)
"""
import os
import numpy as np

import concourse.bass as bass
import concourse.bacc as bacc
import concourse.mybir as mybir
import concourse.tile as tile
from concourse.bass_utils import run_bass_kernel_spmd

# problem constants (hardcoded per contract)
K = 3
B, Ci, H, Wd, Cin = 4, 32, 14, 14, 8
A, N, D = 32, 288, 16
w = 12
P = B * w * w           # 576 positions
G = 16                  # route nodes per PE chunk
CH = N // G             # 18 chunks; G*Cin = 128 = contraction per chunk
AA = A // 8             # capsules per core
PTILES = [(0, 128), (128, 128), (256, 128), (384, 128), (512, 64)]

F32 = mybir.dt.float32
F16 = mybir.dt.float16

# set KERNEL_ROUTE_F32=1 to run the routing phase in full f32 (slower, exact)
ROUTE_DT = F32 if os.environ.get("KERNEL_ROUTE_F32") else F16

LAST_RESULT = None  # BassKernelResults of the most recent run (for profiling)

_prog_cache = {}


def _squash(nc, sp, s, out_f, pt):
    """out_f = squash(s) over the free axis (D).  s, out_f: [128, D] f32."""
    junk = sp.tile([128, D], F32, tag="sq_junk")
    sn = sp.tile([128, 1], F32, tag="sq_sn")
    nc.vector.tensor_mul(junk[:pt], s[:pt], s[:pt])
    nc.vector.tensor_reduce(out=sn[:pt], in_=junk[:pt],
                            axis=mybir.AxisListType.X, op=mybir.AluOpType.add)
    rt = sp.tile([128, 1], F32, tag="sq_rt")
    nc.scalar.sqrt(rt[:pt], sn[:pt])                      # sqrt(sn) on ACT
    u = sp.tile([128, 1], F32, tag="sq_u")
    nc.vector.tensor_scalar_add(u[:pt], sn[:pt], 1.0)     # 1+sn
    v = sp.tile([128, 1], F32, tag="sq_v")
    nc.vector.tensor_mul(v[:pt], u[:pt], rt[:pt])         # (1+sn)*sqrt(sn)
    r = sp.tile([128, 1], F32, tag="sq_r")
    nc.vector.reciprocal(r[:pt], v[:pt])
    f = sp.tile([128, 1], F32, tag="sq_f")
    nc.vector.tensor_mul(f[:pt], sn[:pt], r[:pt])         # sn/((1+sn)*sqrt(sn))
    nc.vector.tensor_scalar_mul(out_f[:pt], s[:pt], f[:pt])


def _build_program():
    key = ("v1", str(ROUTE_DT))
    if key in _prog_cache:
        return _prog_cache[key]

    nc = bacc.Bacc()   # Bacc lowering splits sync waits (HW 1-wait/inst limit)
    xwt_d = nc.dram_tensor("xwt", [128, CH, P], F32, kind="ExternalInput")
    wbd_d = nc.dram_tensor("wbd", [AA, 128, CH, G * D], F32, kind="ExternalInput")
    wde_d = nc.dram_tensor("wde", [128, AA, CH, D], F32, kind="ExternalInput")
    brep_d = nc.dram_tensor("brep", [128, AA, D], F32, kind="ExternalInput")
    out_d = nc.dram_tensor("out", [AA, P, D], F32, kind="ExternalOutput")

    with tile.TileContext(nc) as tc:
        with (
            tc.tile_pool(name="const", bufs=1) as cp,
            tc.tile_pool(name="wbd", bufs=2) as wp,
            tc.tile_pool(name="p1", bufs=2) as pp,
            tc.tile_pool(name="tmp", bufs=2) as tp,
            tc.tile_pool(name="small", bufs=2) as sp,
            tc.tile_pool(name="psum_p", bufs=4, space="PSUM") as qp,
            tc.tile_pool(name="psum_s", bufs=2, space="PSUM") as qs,
        ):
            xwt = cp.tile([128, CH, P], F32)
            nc.sync.dma_start(xwt[:], xwt_d[:])
            wde = cp.tile([128, AA, CH, D], F32)
            nc.sync.dma_start(wde[:], wde_d[:])
            brep = cp.tile([128, AA, D], F32)
            nc.sync.dma_start(brep[:], brep_d[:])

            for a in range(AA):
                wbd = wp.tile([128, CH, G * D], F32)
                nc.sync.dma_start(wbd[:], wbd_d[a])
                for (p0, pt) in PTILES:
                    # ---- phase A: priors + s0 numerator on the PE
                    P1 = pp.tile([128, N, D], ROUTE_DT)
                    ps0 = qs.tile([128, D], F32)
                    # s0 numerator: one uninterrupted PSUM accumulation chain
                    for ch in range(CH):
                        nc.tensor.matmul(ps0[:pt], xwt[:, ch, p0:p0 + pt],
                                         wde[:, a, ch, :],
                                         start=(ch == 0), stop=(ch == CH - 1))
                    for ch in range(CH):
                        st = xwt[:, ch, p0:p0 + pt]            # lhsT [128, pt]
                        ps = qp.tile([128, G * D], F32)
                        nc.tensor.matmul(ps[:pt], st, wbd[:, ch, :],
                                         start=True, stop=True)
                        # PSUM -> SBUF (fp16 cast) on ACT
                        nc.scalar.copy(P1[:pt, ch * G:(ch + 1) * G, :],
                                       ps[:pt].rearrange("p (g d) -> p g d", g=G))

                    # ---- iter 0: uniform probs
                    s = sp.tile([128, D], F32, tag="s")
                    nc.vector.scalar_tensor_tensor(
                        out=s[:pt], in0=ps0[:pt], scalar=1.0 / N,
                        in1=brep[:pt, a, :],
                        op0=mybir.AluOpType.mult, op1=mybir.AluOpType.add)
                    out_f = sp.tile([128, D], F32, tag="out_f")
                    _squash(nc, sp, s, out_f, pt)

                    logits = sp.tile([128, N], F32, tag="logits")
                    for it in (1, 2):
                        out_h = sp.tile([128, D], ROUTE_DT, tag="out_h")
                        nc.vector.tensor_copy(out_h[:pt], out_f[:pt])
                        # g[p,n] = sum_d P1 * out
                        tmp_g = tp.tile([128, N, D], ROUTE_DT, tag="tmp_g")
                        nc.vector.tensor_mul(
                            tmp_g[:pt], P1[:pt],
                            out_h[:pt, None, :].broadcast_to([pt, N, D]))
                        if it == 1:
                            nc.vector.tensor_reduce(
                                out=logits[:pt], in_=tmp_g[:pt],
                                axis=mybir.AxisListType.X, op=mybir.AluOpType.add)
                        else:
                            gb = sp.tile([128, N], F32, tag="gb")
                            nc.vector.tensor_reduce(
                                out=gb[:pt], in_=tmp_g[:pt],
                                axis=mybir.AxisListType.X, op=mybir.AluOpType.add)
                            nc.vector.tensor_add(logits[:pt], logits[:pt], gb[:pt])
                        # softmax (unnormalized): e = exp(logits - max), se = sum e
                        mx = sp.tile([128, 1], F32, tag="mx")
                        nc.vector.tensor_reduce(
                            out=mx[:pt], in_=logits[:pt],
                            axis=mybir.AxisListType.X, op=mybir.AluOpType.max)
                        nmx = sp.tile([128, 1], F32, tag="nmx")
                        nc.vector.tensor_scalar_mul(nmx[:pt], mx[:pt], -1.0)
                        e = sp.tile([128, N], ROUTE_DT, tag="e")
                        se = sp.tile([128, 1], F32, tag="se")
                        nc.scalar.activation(
                            e[:pt], logits[:pt], mybir.ActivationFunctionType.Exp,
                            bias=nmx[:pt], scale=1.0, accum_out=se[:pt])
                        rc = sp.tile([128, 1], F32, tag="rc")
                        nc.vector.reciprocal(rc[:pt], se[:pt])
                        # s[p,d] = (sum_n e * P1) / se + bias
                        tmp_s = tp.tile([128, D, N], ROUTE_DT, tag="tmp_s")
                        nc.vector.tensor_mul(
                            tmp_s[:pt], P1[:pt].transpose([0, 2, 1]),
                            e[:pt, None, :].broadcast_to([pt, D, N]))
                        sr = sp.tile([128, D], F32, tag="sr")
                        nc.vector.tensor_reduce(
                            out=sr[:pt], in_=tmp_s[:pt],
                            axis=mybir.AxisListType.X, op=mybir.AluOpType.add)
                        s = sp.tile([128, D], F32, tag="s")
                        nc.vector.scalar_tensor_tensor(
                            out=s[:pt], in0=sr[:pt], scalar=rc[:pt],
                            in1=brep[:pt, a, :],
                            op0=mybir.AluOpType.mult, op1=mybir.AluOpType.add)
                        out_f = sp.tile([128, D], F32, tag="out_f")
                        _squash(nc, sp, s, out_f, pt)

                    nc.sync.dma_start(out_d[a, p0:p0 + pt, :], out_f[:pt])

    nc.finalize()   # runs Bacc lowering (reg alloc, sync-wait splitting)
    _prog_cache[key] = nc
    return nc


def _host_prep(x, route_weights, bias):
    x = np.ascontiguousarray(x, dtype=np.float32)
    Wfull = np.ascontiguousarray(route_weights, dtype=np.float32)
    bias = np.ascontiguousarray(bias, dtype=np.float32)

    # im2col: xw[p, n, c], node ordering (ci, ki, kj) as in torch .view
    xw = np.empty((B, w, w, Ci, K, K, Cin), np.float32)
    for ki in range(K):
        for kj in range(K):
            xw[:, :, :, :, ki, kj, :] = (
                x[:, :, ki:ki + w, kj:kj + w, :].transpose(0, 2, 3, 1, 4))
    xw = xw.reshape(P, N, Cin)

    xw4 = xw.reshape(P, CH, G, Cin)
    xwt_h = np.ascontiguousarray(xw4.transpose(2, 3, 1, 0)).reshape(128, CH, P)

    Wn = Wfull.reshape(A, CH, G, Cin, D)
    wbd_full = np.zeros((A, G, Cin, CH, G, D), np.float32)
    for g in range(G):
        # [A, CH, Cin, D] -> [A, Cin, CH, D]
        wbd_full[:, g, :, :, g, :] = Wn[:, :, g, :, :].transpose(0, 2, 1, 3)
    wbd_h = wbd_full.reshape(A, 128, CH, G * D)

    wde_h = np.ascontiguousarray(Wn.transpose(2, 3, 0, 1, 4)).reshape(128, A, CH, D)
    brep_h = np.ascontiguousarray(np.broadcast_to(bias[None], (128, A, D)))

    in_maps = []
    for k in range(8):
        a0 = k * AA
        in_maps.append({
            "xwt": xwt_h,
            "wbd": np.ascontiguousarray(wbd_h[a0:a0 + AA]),
            "wde": np.ascontiguousarray(wde_h[:, a0:a0 + AA]),
            "brep": np.ascontiguousarray(brep_h[:, a0:a0 + AA]),
        })
    return in_maps


def kernel(x, route_weights, bias):
    global LAST_RESULT
    nc = _build_program()
    in_maps = _host_prep(x, route_weights, bias)
    trace = bool(os.environ.get("KERNEL_TRACE"))
    res = run_bass_kernel_spmd(nc, in_maps, list(range(8)), trace=trace)
    LAST_RESULT = res
    full = np.stack([res.results[k]["out"] for k in range(8)])  # [8, AA, P, D]
    full = full.reshape(A, B, w, w, D)
    return np.ascontiguousarray(full.transpose(1, 0, 2, 3, 4))
